# revision 32
# baseline (speedup 1.0000x reference)
"""MicroLlama forward pass on 8 Trainium2 NeuronCores.

Strategy: sequence-parallel transformer layers + vocab-parallel output head,
software-pipelined at the sequence-slot level.
 - The residual stream h stays column(sequence)-sharded: core c owns s-tiles
   {c, 15-c} (128 positions each) for causal load balance. Slot E = tile c,
   slot L = tile 15-c.
 - Per (layer, slot) there is ONE combined K+V AllGather (393 KB per rank).
   Slot E's attention only needs the E-halves of K/V (ktiles 0..7), so it is
   gated on AG-E alone; slot L needs both. Each AG is launched right after
   its slot's qkv and overlaps the *other* slot's attention+FFN, keeping the
   PE busy (and HAM-warm) through the collective.
 - After the final norm, two per-slot AllGathers replicate h^T; each core
   computes its own 2 s-tiles' logits during the AGs, then the other 14.
 - Weights are host-pre-transposed and cast to bf16; all matmuls run bf16
   with fp32 PSUM accumulation; the residual stays fp32. Logits are written
   bf16 and widened to f32 on the host.
"""

import numpy as np

try:
    import concourse.bass as bass
except ImportError:  # grading env fallback
    import sys

    sys.path.insert(0, "/opt/trn_rl_repo")
    import concourse.bass as bass

import ml_dtypes
import concourse.tile as tile
import concourse.mybir as mybir
from concourse import bacc
from concourse.bass_utils import run_bass_kernel_spmd
from concourse.masks import make_identity

BF16NP = ml_dtypes.bfloat16
F32 = mybir.dt.float32
BF = mybir.dt.bfloat16

VOCAB, DIM, NL, NH, SEQ = 32000, 768, 4, 12, 2048
HD = 64
HIDDEN = 2042
HIDP = 2048
NCORES = 8
NT = SEQ // 128          # 16 s-tiles
SLOC = 256               # per-core sequence columns
DC = DIM // 128          # 6 contraction chunks
HC = HIDP // 128         # 16 hidden chunks
VSH = VOCAB // NCORES    # 4000
VW = 780                 # V block width: 12 heads x (64 + ones col)
THETA = 10000.0
EPS = 1e-6
SCALE = 0.125            # 1/sqrt(64)
RG = [list(range(NCORES))]
KVW = DIM * 128 + 128 * VW   # flat elems per rank in the KV AG buffer

QKV_CHUNKS = [(768, 512), (1280, 256), (1536, 512), (2048, 256), (0, 512), (512, 256)]
TWO_CHUNKS = [(0, 512), (512, 256)]  # for 768-wide outputs


def _build_program():
    import os
    global NL_BUILD
    NL_BUILD = int(os.environ.get("K_NL", NL))
    nc = bacc.Bacc("TRN2", target_bir_lowering=False, debug=False,
                   enable_asserts=True, num_devices=NCORES)
    io = {}
    io["x0"] = nc.dram_tensor("x0", [SLOC, DIM], F32, kind="ExternalInput").ap()
    io["wqkv"] = nc.dram_tensor("wqkv", [NL, DIM, 3 * DIM], BF, kind="ExternalInput").ap()
    io["wot"] = nc.dram_tensor("wot", [NL, DIM, DIM], BF, kind="ExternalInput").ap()
    io["w13"] = nc.dram_tensor("w13", [NL, DIM, 2 * HIDP], BF, kind="ExternalInput").ap()
    io["w2t"] = nc.dram_tensor("w2t", [NL, HIDP, DIM], BF, kind="ExternalInput").ap()
    io["owt"] = nc.dram_tensor("owt", [DIM, VSH], BF, kind="ExternalInput").ap()
    io["cosn"] = nc.dram_tensor("cosn", [SLOC, HD], BF, kind="ExternalInput").ap()
    io["sinn"] = nc.dram_tensor("sinn", [SLOC, HD], BF, kind="ExternalInput").ap()
    io["mask0"] = nc.dram_tensor("mask0", [128, 8 * 256], BF, kind="ExternalInput").ap()
    io["mask1"] = nc.dram_tensor("mask1", [128, 8 * 128], BF, kind="ExternalInput").ap()
    io["logits"] = nc.dram_tensor("logits", [SEQ, VSH], BF, kind="ExternalOutput").ap()

    with tile.TileContext(nc) as tc:
        _body(tc, io)
    nc.compile()
    return nc


def _body(tc, io):
    nc = tc.nc
    from contextlib import ExitStack

    ctx = ExitStack()
    sing = ctx.enter_context(tc.tile_pool(name="sing", bufs=1))
    wq_p = ctx.enter_context(tc.tile_pool(name="wq_p", bufs=6))
    wb_p = ctx.enter_context(tc.tile_pool(name="wb_p", bufs=6))
    tmp = ctx.enter_context(tc.tile_pool(name="tmp", bufs=2))
    tmpb = ctx.enter_context(tc.tile_pool(name="tmpb", bufs=2))
    exp_p = ctx.enter_context(tc.tile_pool(name="exp_p", bufs=2))
    mm = ctx.enter_context(tc.tile_pool(name="mm", bufs=2, space="PSUM"))
    ovp = ctx.enter_context(tc.tile_pool(name="ovp", bufs=2, space="PSUM"))
    sc = ctx.enter_context(tc.tile_pool(name="sc", bufs=2, space="PSUM"))
    dram = ctx.enter_context(tc.tile_pool(name="dram", bufs=2, space="DRAM"))

    # ---- persistent state ----
    h = sing.tile([128, 2, DIM], F32)
    kTf = sing.tile([128, DC, SEQ], BF, tag="kTf")   # gathered K^T, rank-major cols
    v_aug = sing.tile([128, NT, VW], BF)             # V rows + ones col per head
    qT = sing.tile([128, DC, SLOC], BF)
    koT = sing.tile([128, DC, SLOC], BF, tag="koT")  # own K^T (pre-gather)
    oT = sing.tile([128, DC, SLOC], BF)              # attention out, transposed
    vpad = sing.tile([128, 2, VW], BF)               # own V in 65-stride + ones
    cos_s = sing.tile([128, 2, HD], BF)
    sin_s = sing.tile([128, 2, HD], BF)
    m0 = sing.tile([128, 8 * 256], BF)
    m1 = sing.tile([128, 8 * 128], BF)
    ident = sing.tile([128, 128], BF)
    spillL = sing.tile([128, NH * 128], BF)    # L-query PV partial spill

    make_identity(nc, ident)
    nc.vector.memset(vpad, 1.0)  # ones columns persist; V cols overwritten
    eps_t = sing.tile([128, 1], F32, name="eps_t")
    nc.vector.memset(eps_t, EPS)
    nc.sync.dma_start(out=m0, in_=io["mask0"])
    nc.sync.dma_start(out=m1, in_=io["mask1"])
    for st in (0, 1):
        nc.sync.dma_start(out=h[:, st, :], in_=io["x0"][128 * st:128 * (st + 1), :])
        nc.sync.dma_start(out=cos_s[:, st, :], in_=io["cosn"][128 * st:128 * (st + 1), :])
        nc.sync.dma_start(out=sin_s[:, st, :], in_=io["sinn"][128 * st:128 * (st + 1), :])

    def rmsnorm_cast(dst_bf, src_ap):
        """dst_bf[128, DIM] (bf16) = rmsnorm(src_ap[128, DIM] f32)."""
        xsq = tmp.tile([128, DIM], BF, tag="scr1", bufs=1, name="xsq")
        nc.vector.tensor_mul(out=xsq, in0=src_ap, in1=src_ap)
        ssum = tmp.tile([128, 1], F32, tag="ssum", name="ssum")
        nc.vector.reduce_sum(out=ssum, in_=xsq, axis=mybir.AxisListType.X)
        rstd = tmp.tile([128, 1], F32, tag="rstd", name="rstd")
        nc.scalar.activation(out=rstd, in_=ssum,
                             func=mybir.ActivationFunctionType.Sqrt,
                             bias=eps_t, scale=1.0 / DIM)
        nc.vector.reciprocal(out=rstd, in_=rstd)
        nc.vector.tensor_scalar_mul(out=dst_bf, in0=src_ap, scalar1=rstd)

    def transpose128(dst_ap, src_ap):
        """dst[128,128] (bf16 sbuf slice) = src[128,128] (bf16 sbuf).T via PE."""
        pt = mm.tile([128, 512], BF, tag="mm", name="tp")
        nc.tensor.transpose(pt[:, 0:128], src_ap, ident)
        nc.vector.tensor_copy(out=dst_ap, in_=pt[:, 0:128])

    def rope(dst_bf, src_ap, st):
        """dst[128, DIM] bf16 = rope(src[128, DIM] bf16) for s-tile slot st."""
        rot = tmp.tile([128, DIM], BF, tag="scr1", bufs=1, name="rot")
        sp = src_ap.rearrange("p (n two) -> p n two", two=2)
        rp = rot.rearrange("p (n two) -> p n two", two=2)
        nc.vector.tensor_copy(out=rp[:, :, 0], in_=sp[:, :, 1])
        nc.vector.tensor_copy(out=rp[:, :, 1], in_=sp[:, :, 0])
        csb = cos_s[:, st, :].unsqueeze(1).broadcast_to([128, NH, HD])
        snb = sin_s[:, st, :].unsqueeze(1).broadcast_to([128, NH, HD])
        sv = src_ap.rearrange("p (nh e) -> p nh e", e=HD)
        nc.vector.tensor_mul(out=dst_bf.rearrange("p (nh e) -> p nh e", e=HD),
                             in0=sv, in1=csb)
        nc.vector.tensor_mul(out=rot.rearrange("p (nh e) -> p nh e", e=HD),
                             in0=rot.rearrange("p (nh e) -> p nh e", e=HD),
                             in1=snb)
        nc.vector.tensor_add(out=dst_bf, in0=dst_bf, in1=rot)

    wqkv_sb = {}       # layer -> [128, DC, 3*DIM] sbuf tile
    wo_sb = {}         # layer -> [128, DC, DIM]
    w2_sb = {}         # layer -> [128, HC, DIM]
    kv_out_t = {}      # (layer, st) -> shared AG output

    def load_wqkv(l):
        wt = wq_p.tile([128, DC, 3 * DIM], BF, tag="wqkv", bufs=1, name=f"wqkv{l}")
        nc.sync.dma_start(
            out=wt, in_=io["wqkv"][l].rearrange("(dc p) c -> p dc c", p=128))
        wqkv_sb[l] = wt

    def load_wo_w2(l):
        wo = wq_p.tile([128, DC, DIM], BF, tag="wo", bufs=1, name=f"wo{l}")
        nc.sync.dma_start(
            out=wo, in_=io["wot"][l].rearrange("(dc p) c -> p dc c", p=128))
        wo_sb[l] = wo
        w2 = wq_p.tile([128, HC, DIM], BF, tag="w2", bufs=1, name=f"w2{l}")
        nc.sync.dma_start(
            out=w2, in_=io["w2t"][l].rearrange("(hc p) c -> p hc c", p=128))
        w2_sb[l] = w2

    def qkv_gen(l, st):
        """qkv for (layer l, slot st) + combined K+V AllGather launch."""
        sl = slice(128 * st, 128 * (st + 1))
        xhT = tmpb.tile([128, DC, 128], BF, tag="xhT", bufs=1, name=f"xhT{l}_{st}")
        xhb = tmpb.tile([128, DIM], BF, tag="qkn", bufs=1, name=f"xhb{l}_{st}")
        rmsnorm_cast(xhb, h[:, st, :])
        yield
        for dc in range(DC):
            transpose128(xhT[:, dc, :], xhb[:, 128 * dc:128 * (dc + 1)])
        yield
        qkn = tmpb.tile([128, DIM], BF, tag="qkn", bufs=1, name=f"qkn{l}_{st}")
        vps = vpad[:, st, :].rearrange("p (nh e) -> p nh e", e=65)[:, :, 0:64]

        def mm_chunk(c0, cw):
            pt = mm.tile([128, 512], F32, tag="mm", name=f"qkvp{l}_{st}_{c0}")
            for dc in range(DC):
                nc.tensor.matmul(pt[:, :cw], xhT[:, dc, :],
                                 wqkv_sb[l][:, dc, c0:c0 + cw],
                                 start=(dc == 0), stop=(dc == DC - 1))
            if c0 >= 1536:  # V chunk: write strided into vpad (ones survive)
                h0 = (c0 - 1536) // 64
                nc.vector.tensor_copy(
                    out=vps[:, h0:h0 + cw // 64, :],
                    in_=pt[:, :cw].rearrange("p (nh e) -> p nh e", e=64))
            else:
                nc.vector.tensor_copy(out=qkn[:, c0 % DIM:c0 % DIM + cw],
                                      in_=pt[:, :cw])

        for (c0, cw) in QKV_CHUNKS[0:2]:   # K
            mm_chunk(c0, cw)
        yield
        kr = tmpb.tile([128, DIM], BF, tag="qkr", bufs=1, name=f"kr{l}_{st}")
        rope(kr, qkn[:, 0:DIM], st)
        for dc in range(DC):
            transpose128(koT[:, dc, sl], kr[:, 128 * dc:128 * (dc + 1)])
        yield
        for (c0, cw) in QKV_CHUNKS[2:4]:   # V
            mm_chunk(c0, cw)
        yield

        # combined K^T + V bounce + AllGather
        kv_in = dram.tile([KVW], BF, tag="kv_in", name=f"kvi{l}_{st}")
        nc.sync.dma_start(
            out=kv_in[0:DIM * 128].rearrange("(dc p s) -> p dc s", p=128, s=128),
            in_=koT[:, :, sl])
        nc.sync.dma_start(
            out=kv_in[DIM * 128:].rearrange("(p w) -> p w", p=128),
            in_=vpad[:, st, :])
        kv_out = dram.tile([NCORES, KVW], BF, tag="kv_out",
                           addr_space="Shared", name=f"kvo{l}_{st}")
        nc.gpsimd.collective_compute("AllGather", mybir.AluOpType.bypass,
                                     replica_groups=RG,
                                     ins=[kv_in.opt()], outs=[kv_out.opt()])
        kv_out_t[(l, st)] = kv_out

        # Q chunks + rope + transpose (off the AG critical path)
        for (c0, cw) in QKV_CHUNKS[4:6]:   # Q
            mm_chunk(c0, cw)
        yield
        qr = tmpb.tile([128, DIM], BF, tag="qkr", bufs=1, name=f"qr{l}_{st}")
        rope(qr, qkn[:, 0:DIM], st)
        for dc in range(DC):
            transpose128(qT[:, dc, sl], qr[:, 128 * dc:128 * (dc + 1)])

    def unbounce(l, st):
        """Scatter AG(l, st) output into kTf + v_aug (rank-major v index)."""
        kv_out = kv_out_t.pop((l, st))
        # kTf cols for rank r live at 256*r + 128*st (4D DMA unsupported)
        for r in range(NCORES):
            nc.sync.dma_start(
                out=kTf[:, :, 256 * r + 128 * st:256 * r + 128 * (st + 1)],
                in_=kv_out[r, 0:DIM * 128].rearrange("(dc p s) -> p dc s",
                                                     p=128, s=128))
        # v_aug block index: rank-major [st*8 + r]
        nc.sync.dma_start(
            out=v_aug[:, 8 * st:8 * (st + 1), :],
            in_=kv_out[:, DIM * 128:].rearrange("r (p w) -> p r w", p=128))

    def epilogue(ov_ap, l, st, hh):
        dch, offh = divmod(hh, 2)
        off = 64 * offh
        sl = slice(128 * st, 128 * (st + 1))
        rbc = tmp.tile([64, 128], F32, tag="rbc", bufs=2, name=f"rbc{l}_{st}_{hh}")
        nc.vector.reciprocal(out=rbc[0:1, :], in_=ov_ap[64:65, 0:128])
        nc.gpsimd.partition_broadcast(out_ap=rbc, in_ap=rbc[0:1, :])
        nc.vector.tensor_mul(out=oT[off:off + 64, dch, sl],
                             in0=ov_ap[0:64, 0:128], in1=rbc)

    def attn_gen(l, st):
        """Attention + wo + residual for (layer l, slot st). Yields per head
        so the emitter can interleave independent PE work into its bubbles.

        st=0 (after AG-E): scores for ktiles 0..7 against BOTH slots'
        queries (256 moving cols per stationary load). The E-half finishes
        (mask + PV + normalize); the L-half PV partial (ktiles 0..7 are
        always fully allowed for L queries, no mask) spills to SBUF.
        st=1 (after AG-L): ktiles 8..15 vs L queries only, then adds the
        spilled partial back before normalizing.
        """
        sl = slice(128 * st, 128 * (st + 1))
        for hh in range(NH):
            dch, offh = divmod(hh, 2)
            off = 64 * offh
            ov = ovp.tile([128, 512], F32, tag="ov", name=f"ov{l}_{st}_{hh}")
            if st == 0:
                for u in range(2):
                    sp = sc.tile([128, 1024], F32, tag="sc",
                                 name=f"sc{l}_{st}_{hh}_{u}")
                    for ktl in range(4):
                        kt = 4 * u + ktl
                        nc.tensor.matmul(
                            sp[:, 256 * ktl:256 * (ktl + 1)],
                            kTf[off:off + 64, dch, 256 * kt:256 * kt + 128],
                            qT[off:off + 64, dch, 0:256],
                            start=True, stop=True)
                    et = exp_p.tile([128, 1024], BF, tag="et", bufs=3,
                                    name=f"et{l}_{st}_{hh}_{u}")
                    nc.scalar.activation(out=et, in_=sp,
                                         func=mybir.ActivationFunctionType.Exp,
                                         scale=SCALE)
                    # mask: E-half gets the causal mask, L-half all-ones
                    nc.vector.tensor_mul(out=et, in0=et,
                                         in1=m0[:, 1024 * u:1024 * (u + 1)])
                    for ktl in range(4):
                        kt = 4 * u + ktl
                        nc.tensor.matmul(ov[0:65, 0:256],
                                         v_aug[:, kt, 65 * hh:65 * (hh + 1)],
                                         et[:, 256 * ktl:256 * (ktl + 1)],
                                         start=(kt == 0), stop=(kt == 7))
                epilogue(ov[:, 0:128], l, st, hh)
                nc.vector.tensor_copy(out=spillL[0:65, 128 * hh:128 * (hh + 1)],
                                      in_=ov[0:65, 128:256])
                yield
            else:
                sp = sc.tile([128, 1024], F32, tag="sc",
                             name=f"sc{l}_{st}_{hh}_1")
                for ktl in range(8):
                    kt = 8 + ktl
                    col = 256 * (15 - kt) + 128
                    nc.tensor.matmul(
                        sp[:, 128 * ktl:128 * (ktl + 1)],
                        kTf[off:off + 64, dch, col:col + 128],
                        qT[off:off + 64, dch, sl],
                        start=True, stop=True)
                et = exp_p.tile([128, 1024], BF, tag="et", bufs=3,
                                name=f"et{l}_{st}_{hh}_1")
                nc.scalar.activation(out=et, in_=sp,
                                     func=mybir.ActivationFunctionType.Exp,
                                     scale=SCALE)
                nc.vector.tensor_mul(out=et, in0=et, in1=m1[:, 0:1024])
                for ktl in range(8):
                    kt = 8 + ktl
                    vj = 8 + (15 - kt)
                    nc.tensor.matmul(ov[0:65, 0:128],
                                     v_aug[:, vj, 65 * hh:65 * (hh + 1)],
                                     et[:, 128 * ktl:128 * (ktl + 1)],
                                     start=(kt == 8), stop=(kt == 15))
                ovs = tmp.tile([128, 128], F32, tag="ovs", bufs=2, name=f"ovs{l}_{hh}")
                nc.vector.tensor_add(out=ovs[0:65, :], in0=ov[0:65, 0:128],
                                     in1=spillL[0:65, 128 * hh:128 * (hh + 1)])
                epilogue(ovs, l, st, hh)
                yield

        # wo projection + residual for this slot's columns
        pts = {}
        for ci, (c0, cw) in enumerate(TWO_CHUNKS):
            pts[ci] = mm.tile([128, 512], F32, tag="mm", name=f"wop{l}_{st}_{ci}")
        for dc in range(DC):
            for ci, (c0, cw) in enumerate(TWO_CHUNKS):
                nc.tensor.matmul(pts[ci][:, :cw],
                                 oT[:, dc, sl],
                                 wo_sb[l][:, dc, c0:c0 + cw],
                                 start=(dc == 0), stop=(dc == DC - 1))
        for ci, (c0, cw) in enumerate(TWO_CHUNKS):
            nc.vector.tensor_add(out=h[:, st, c0:c0 + cw],
                                 in0=h[:, st, c0:c0 + cw],
                                 in1=pts[ci][:, :cw])

    w13_sb = {}

    def load_w13(l):
        wt = wb_p.tile([128, DC, 2 * HIDP], BF, tag="wbig", bufs=1, name=f"w13{l}")
        nc.sync.dma_start(
            out=wt, in_=io["w13"][l].rearrange("(dc p) c -> p dc c", p=128))
        w13_sb[l] = wt

    def ffn_gen(l, st):
        sl = slice(128 * st, 128 * (st + 1))
        yhT = tmpb.tile([128, DC, 128], BF, tag="xhT", bufs=1, name=f"yhT{l}_{st}")
        yhb = tmpb.tile([128, DIM], BF, tag="qkn", bufs=1, name=f"yhb{l}_{st}")
        rmsnorm_cast(yhb, h[:, st, :])
        yield
        for dc in range(DC):
            transpose128(yhT[:, dc, :], yhb[:, 128 * dc:128 * (dc + 1)])
        yield
        zbT = tmpb.tile([128, HC, 128], BF, tag="zbT", bufs=1, name=f"zbT{l}_{st}")
        for ck in range(4):
            p1 = mm.tile([128, 512], F32, tag="mm", name=f"z1p{l}_{st}_{ck}")
            p3 = mm.tile([128, 512], F32, tag="mm", name=f"z3p{l}_{st}_{ck}")
            for dc in range(DC):
                nc.tensor.matmul(p1, yhT[:, dc, :],
                                 w13_sb[l][:, dc, 512 * ck:512 * (ck + 1)],
                                 start=(dc == 0), stop=(dc == DC - 1))
                nc.tensor.matmul(p3, yhT[:, dc, :],
                                 w13_sb[l][:, dc, HIDP + 512 * ck:HIDP + 512 * (ck + 1)],
                                 start=(dc == 0), stop=(dc == DC - 1))
            sil = tmp.tile([128, 512], BF, tag="scr2", bufs=1,
                           name=f"sil{l}_{st}_{ck}")
            nc.scalar.activation(out=sil, in_=p1,
                                 func=mybir.ActivationFunctionType.Silu)
            zc = tmpb.tile([128, 512], BF, tag="zbc", name=f"zc{l}_{st}_{ck}")
            nc.vector.tensor_mul(out=zc, in0=sil, in1=p3)
            for j in range(4):
                transpose128(zbT[:, 4 * ck + j, :], zc[:, 128 * j:128 * (j + 1)])
            yield
        pts = {}
        for ci, (c0, cw) in enumerate(TWO_CHUNKS):
            pts[ci] = mm.tile([128, 512], F32, tag="mm", name=f"w2p{l}_{st}_{ci}")
        for hc in range(HC):
            for ci, (c0, cw) in enumerate(TWO_CHUNKS):
                nc.tensor.matmul(pts[ci][:, :cw],
                                 zbT[:, hc, :],
                                 w2_sb[l][:, hc, c0:c0 + cw],
                                 start=(hc == 0), stop=(hc == HC - 1))
            if hc % 4 == 3:
                yield
        for ci, (c0, cw) in enumerate(TWO_CHUNKS):
            nc.vector.tensor_add(out=h[:, st, c0:c0 + cw],
                                 in0=h[:, st, c0:c0 + cw],
                                 in1=pts[ci][:, :cw])

    # ---------- final norm / AG / output head helpers ----------
    hnT = sing.tile([128, DC, SLOC], BF, tag="koT", name="hnT")    # alias koT
    hnTf = sing.tile([128, DC, SEQ], BF, tag="kTf", name="hnTf")   # alias kTf
    hf_out_t = {}

    def final_gen(st):
        """Final rmsnorm + per-slot AllGather of h^T."""
        sl = slice(128 * st, 128 * (st + 1))
        fnb = tmpb.tile([128, DIM], BF, tag="qkn", bufs=1, name=f"fnb{st}")
        rmsnorm_cast(fnb, h[:, st, :])
        yield
        for dc in range(DC):
            transpose128(hnT[:, dc, sl], fnb[:, 128 * dc:128 * (dc + 1)])
        yield
        hf_in = dram.tile([DIM * 128], BF, tag="kv_in", name=f"hfi{st}")
        nc.sync.dma_start(
            out=hf_in.rearrange("(dc p s) -> p dc s", p=128, s=128),
            in_=hnT[:, :, sl])
        hf_out = dram.tile([NCORES, DIM * 128], BF, tag="kv_out",
                           addr_space="Shared", name=f"hfo{st}")
        nc.gpsimd.collective_compute("AllGather", mybir.AluOpType.bypass,
                                     replica_groups=RG,
                                     ins=[hf_in.opt()], outs=[hf_out.opt()])
        hf_out_t[st] = hf_out

    def head_block(src_ap, row0, tag):
        """logits[row0:row0+128, :] from src_ap = h^T cols [128, DC, 128]."""
        for half in range(8):
            ldr = tmp.tile([128, 500], BF, tag="ldr", bufs=2,
                           name=f"ldr{tag}_{half}")
            for vq in range(1):
                vc = half
                pt = mm.tile([128, 512], F32, tag="mm", name=f"hd{tag}_{vc}")
                for dc in range(DC):
                    nc.tensor.matmul(pt[:, :500],
                                     src_ap[:, dc, :],
                                     owt_sb[:, dc, 500 * vc:500 * (vc + 1)],
                                     start=(dc == 0), stop=(dc == DC - 1))
                if vc % 2 == 0:
                    nc.vector.tensor_copy(out=ldr[:, 500 * vq:500 * (vq + 1)],
                                          in_=pt[:, :500])
                else:
                    nc.scalar.copy(out=ldr[:, 500 * vq:500 * (vq + 1)],
                                   in_=pt[:, :500])
            nc.sync.dma_start(out=io["logits"][row0:row0 + 128,
                                               500 * half:500 * (half + 1)],
                              in_=ldr)

    # ================= the pipelined program =================
    def drive(*gens):
        live = list(gens)
        while live:
            nxt = []
            for g in live:
                try:
                    next(g)
                    nxt.append(g)
                except StopIteration:
                    pass
            live = nxt

    load_wqkv(0)
    load_wo_w2(0)
    drive(qkv_gen(0, 0))
    drive(qkv_gen(0, 1))
    load_w13(0)

    def e_chain(l):
        yield from ffn_gen(l, 0)
        if l + 1 < NL_BUILD:
            yield from qkv_gen(l + 1, 0)
        else:
            yield from final_gen(0)

    for l in range(NL_BUILD):
        unbounce(l, 0)
        drive(attn_gen(l, 0))
        unbounce(l, 1)
        if l + 1 < NL_BUILD:
            load_wqkv(l + 1)
        # slot-L attention fills the PE bubbles of slot-E's ffn+qkv chain
        drive(e_chain(l), attn_gen(l, 1))
        drive(ffn_gen(l, 1))
        if l + 1 < NL_BUILD:
            load_w13(l + 1)
            load_wo_w2(l + 1)
            drive(qkv_gen(l + 1, 1))
        else:
            drive(final_gen(1))

    # owt load (after last w13 use frees the pool slot)
    owt_sb = wb_p.tile([128, DC, VSH], BF, tag="wbig", bufs=1, name="owt")
    nc.sync.dma_start(
        out=owt_sb, in_=io["owt"].rearrange("(dc p) c -> p dc c", p=128))

    # un-bounce the final AGs and run the head over all 16 s-tiles.
    for st in (0, 1):
        hf_out = hf_out_t.pop(st)
        for r in range(NCORES):
            nc.sync.dma_start(
                out=hnTf[:, :, 256 * r + 128 * st:256 * r + 128 * (st + 1)],
                in_=hf_out[r].rearrange("(dc p s) -> p dc s", p=128, s=128))
        for r in range(NCORES):
            st_out = r if st == 0 else 15 - r
            head_block(hnTf[:, :, 256 * r + 128 * st:256 * r + 128 * (st + 1)],
                       128 * st_out, f"{r}_{st}")
    ctx.close()


_NC_CACHE = None


def _get_program():
    global _NC_CACHE
    if _NC_CACHE is None:
        _NC_CACHE = _build_program()
    return _NC_CACHE


def _prep_inputs(tokens, emb, wq, wk, wv, wo, w1, w2, w3,
                 attn_norm_w, ffn_norm_w, norm_w, out_w):
    f32 = np.float32
    tokens = np.asarray(tokens)
    emb = np.asarray(emb, f32)
    wq, wk, wv, wo = (np.asarray(a, f32) for a in (wq, wk, wv, wo))
    w1, w2, w3 = (np.asarray(a, f32) for a in (w1, w2, w3))
    attn_norm_w = np.asarray(attn_norm_w, f32)
    ffn_norm_w = np.asarray(ffn_norm_w, f32)
    norm_w = np.asarray(norm_w, f32)
    out_w = np.asarray(out_w, f32)

    # shared (rank-independent) weight transforms
    wqkv_l, w13_l, w2t_l, wot_l = [], [], [], []
    for l in range(NL):
        qkvT = np.concatenate([wq[l].T, wk[l].T, wv[l].T], axis=1)
        wqkv_l.append(qkvT * attn_norm_w[l][:, None])
        w1p = np.zeros((DIM, HIDP), f32)
        w3p = np.zeros((DIM, HIDP), f32)
        w1p[:, :HIDDEN] = w1[l].T
        w3p[:, :HIDDEN] = w3[l].T
        w13_l.append(np.concatenate([w1p, w3p], axis=1) * ffn_norm_w[l][:, None])
        w2p = np.zeros((HIDP, DIM), f32)
        w2p[:HIDDEN, :] = w2[l].T
        w2t_l.append(w2p)
        wot_l.append(wo[l].T)
    wqkv_np = np.stack(wqkv_l).astype(BF16NP)
    w13_np = np.stack(w13_l).astype(BF16NP)
    w2t_np = np.stack(w2t_l).astype(BF16NP)
    wot_np = np.stack(wot_l).astype(BF16NP)
    out_w_n = out_w * norm_w[None, :]

    freqs = THETA ** (-np.arange(0, HD, 2, dtype=f32) / HD)  # [32]
    kl = np.arange(128)[:, None]
    ql = np.arange(128)[None, :]

    in_maps = []
    for c in range(NCORES):
        tiles = (c, 15 - c)
        cols = np.concatenate([np.arange(128 * t, 128 * (t + 1)) for t in tiles])
        tok_own = np.asarray(tokens).reshape(-1)[cols].astype(np.int64)
        x0 = emb[tok_own].astype(f32)

        pos = cols.astype(f32)
        ang = pos[:, None] * freqs[None, :]          # [256, 32]
        cosp = np.cos(ang).astype(f32)
        sinp = np.sin(ang).astype(f32)
        cosn = np.repeat(cosp, 2, axis=1).astype(BF16NP)
        sinn = np.stack([-sinp, sinp], axis=2).reshape(SLOC, HD).astype(BF16NP)

        def make_mask(qtile, kt0, nkt):
            blocks = [(128 * kt + kl <= 128 * qtile + ql)
                      for kt in range(kt0, kt0 + nkt)]
            return np.concatenate(blocks, axis=1).astype(BF16NP)

        def make_mask0c(qtile):
            ones = np.ones((128, 128), bool)
            blocks = []
            for kt in range(8):
                blocks.append(128 * kt + kl <= 128 * qtile + ql)
                blocks.append(ones)
            return np.concatenate(blocks, axis=1).astype(BF16NP)

        owt_np = np.ascontiguousarray(
            out_w_n[VSH * c:VSH * (c + 1), :].T).astype(BF16NP)

        in_maps.append({
            "x0": x0,
            "wqkv": wqkv_np, "wot": wot_np, "w13": w13_np, "w2t": w2t_np,
            "owt": owt_np,
            "cosn": cosn, "sinn": sinn,
            "mask0": make_mask0c(c), "mask1": make_mask(15 - c, 8, 8),
        })
    return in_maps


def kernel(**inputs):
    nc = _get_program()
    in_maps = _prep_inputs(**inputs)
    res = run_bass_kernel_spmd(nc, in_maps, core_ids=list(range(NCORES)))
    shards = [np.asarray(res.results[c]["logits"], dtype=np.float32)
              for c in range(NCORES)]
    full = np.concatenate(shards, axis=1)  # [2048, 32000]
    return full[None]


# revision 33
# speedup vs baseline: 1.0243x; 1.0243x over previous
"""MicroLlama forward pass on 8 Trainium2 NeuronCores.

Strategy: sequence-parallel transformer layers + vocab-parallel output head,
software-pipelined at the sequence-slot level.
 - The residual stream h stays column(sequence)-sharded: core c owns s-tiles
   {c, 15-c} (128 positions each) for causal load balance. Slot E = tile c,
   slot L = tile 15-c.
 - Per (layer, slot) there is ONE combined K+V AllGather (393 KB per rank).
   Slot E's attention only needs the E-halves of K/V (ktiles 0..7), so it is
   gated on AG-E alone; slot L needs both. Each AG is launched right after
   its slot's qkv and overlaps the *other* slot's attention+FFN, keeping the
   PE busy (and HAM-warm) through the collective.
 - After the final norm, two per-slot AllGathers replicate h^T; each core
   computes its own 2 s-tiles' logits during the AGs, then the other 14.
 - Weights are host-pre-transposed and cast to bf16; all matmuls run bf16
   with fp32 PSUM accumulation; the residual stays fp32. Logits are written
   bf16 and widened to f32 on the host.
"""

import numpy as np

try:
    import concourse.bass as bass
except ImportError:  # grading env fallback
    import sys

    sys.path.insert(0, "/opt/trn_rl_repo")
    import concourse.bass as bass

import ml_dtypes
import concourse.tile as tile
import concourse.mybir as mybir
from concourse import bacc
from concourse.bass_utils import run_bass_kernel_spmd
from concourse.masks import make_identity

BF16NP = ml_dtypes.bfloat16
F32 = mybir.dt.float32
BF = mybir.dt.bfloat16

VOCAB, DIM, NL, NH, SEQ = 32000, 768, 4, 12, 2048
HD = 64
HIDDEN = 2042
HIDP = 2048
NCORES = 8
NT = SEQ // 128          # 16 s-tiles
SLOC = 256               # per-core sequence columns
DC = DIM // 128          # 6 contraction chunks
HC = HIDP // 128         # 16 hidden chunks
VSH = VOCAB // NCORES    # 4000
VW = 780                 # V block width: 12 heads x (64 + ones col)
THETA = 10000.0
EPS = 1e-6
SCALE = 0.125            # 1/sqrt(64)
RG = [list(range(NCORES))]
KVW = DIM * 128 + 128 * VW   # flat elems per rank in the KV AG buffer

QKV_CHUNKS = [(768, 512), (1280, 256), (1536, 512), (2048, 256), (0, 512), (512, 256)]
TWO_CHUNKS = [(0, 512), (512, 256)]  # for 768-wide outputs


def _build_program():
    import os
    global NL_BUILD
    NL_BUILD = int(os.environ.get("K_NL", NL))
    nc = bacc.Bacc("TRN2", target_bir_lowering=False, debug=False,
                   enable_asserts=True, num_devices=NCORES)
    io = {}
    io["x0"] = nc.dram_tensor("x0", [SLOC, DIM], F32, kind="ExternalInput").ap()
    io["wqkv"] = nc.dram_tensor("wqkv", [NL, DIM, 3 * DIM], BF, kind="ExternalInput").ap()
    io["wot"] = nc.dram_tensor("wot", [NL, DIM, DIM], BF, kind="ExternalInput").ap()
    io["w13"] = nc.dram_tensor("w13", [NL, DIM, 2 * HIDP], BF, kind="ExternalInput").ap()
    io["w2t"] = nc.dram_tensor("w2t", [NL, HIDP, DIM], BF, kind="ExternalInput").ap()
    io["owt"] = nc.dram_tensor("owt", [DIM, VSH], BF, kind="ExternalInput").ap()
    io["cosn"] = nc.dram_tensor("cosn", [SLOC, HD], BF, kind="ExternalInput").ap()
    io["sinn"] = nc.dram_tensor("sinn", [SLOC, HD], BF, kind="ExternalInput").ap()
    io["mask0"] = nc.dram_tensor("mask0", [128, 8 * 256], BF, kind="ExternalInput").ap()
    io["mask1"] = nc.dram_tensor("mask1", [128, 8 * 128], BF, kind="ExternalInput").ap()
    io["logits"] = nc.dram_tensor("logits", [SEQ, VSH], BF, kind="ExternalOutput").ap()

    with tile.TileContext(nc) as tc:
        _body(tc, io)
    nc.compile()
    return nc


def _body(tc, io):
    nc = tc.nc
    from contextlib import ExitStack

    ctx = ExitStack()
    sing = ctx.enter_context(tc.tile_pool(name="sing", bufs=1))
    wq_p = ctx.enter_context(tc.tile_pool(name="wq_p", bufs=6))
    wb_p = ctx.enter_context(tc.tile_pool(name="wb_p", bufs=6))
    tmp = ctx.enter_context(tc.tile_pool(name="tmp", bufs=2))
    tmpb = ctx.enter_context(tc.tile_pool(name="tmpb", bufs=2))
    exp_p = ctx.enter_context(tc.tile_pool(name="exp_p", bufs=2))
    mm = ctx.enter_context(tc.tile_pool(name="mm", bufs=2, space="PSUM"))
    ovp = ctx.enter_context(tc.tile_pool(name="ovp", bufs=2, space="PSUM"))
    sc = ctx.enter_context(tc.tile_pool(name="sc", bufs=2, space="PSUM"))
    dram = ctx.enter_context(tc.tile_pool(name="dram", bufs=2, space="DRAM"))

    # ---- persistent state ----
    h = sing.tile([128, 2, DIM], F32)
    kTf = sing.tile([128, DC, SEQ], BF, tag="kTf")   # gathered K^T, rank-major cols
    v_aug = sing.tile([128, NT, VW], BF)             # V rows + ones col per head
    qT = sing.tile([128, DC, SLOC], BF)
    koT = sing.tile([128, DC, SLOC], BF, tag="koT")  # own K^T (pre-gather)
    oT = sing.tile([128, DC, SLOC], BF)              # attention out, transposed
    vpad = sing.tile([128, 2, VW], BF)               # own V in 65-stride + ones
    cos_s = sing.tile([128, 2, HD], BF)
    sin_s = sing.tile([128, 2, HD], BF)
    m0 = sing.tile([128, 8 * 256], BF)
    m1 = sing.tile([128, 8 * 128], BF)
    ident = sing.tile([128, 128], BF)

    make_identity(nc, ident)
    nc.vector.memset(vpad, 1.0)  # ones columns persist; V cols overwritten
    eps_t = sing.tile([128, 1], F32, name="eps_t")
    nc.vector.memset(eps_t, EPS)
    nc.sync.dma_start(out=m0, in_=io["mask0"])
    nc.sync.dma_start(out=m1, in_=io["mask1"])
    for st in (0, 1):
        nc.sync.dma_start(out=h[:, st, :], in_=io["x0"][128 * st:128 * (st + 1), :])
        nc.sync.dma_start(out=cos_s[:, st, :], in_=io["cosn"][128 * st:128 * (st + 1), :])
        nc.sync.dma_start(out=sin_s[:, st, :], in_=io["sinn"][128 * st:128 * (st + 1), :])

    def rmsnorm_cast(dst_bf, src_ap):
        """dst_bf[128, DIM] (bf16) = rmsnorm(src_ap[128, DIM] f32)."""
        xsq = tmp.tile([128, DIM], BF, tag="scr1", bufs=1, name="xsq")
        nc.vector.tensor_mul(out=xsq, in0=src_ap, in1=src_ap)
        ssum = tmp.tile([128, 1], F32, tag="ssum", name="ssum")
        nc.vector.reduce_sum(out=ssum, in_=xsq, axis=mybir.AxisListType.X)
        rstd = tmp.tile([128, 1], F32, tag="rstd", name="rstd")
        nc.scalar.activation(out=rstd, in_=ssum,
                             func=mybir.ActivationFunctionType.Sqrt,
                             bias=eps_t, scale=1.0 / DIM)
        nc.vector.reciprocal(out=rstd, in_=rstd)
        nc.vector.tensor_scalar_mul(out=dst_bf, in0=src_ap, scalar1=rstd)

    def transpose128(dst_ap, src_ap):
        """dst[128,128] (bf16 sbuf slice) = src[128,128] (bf16 sbuf).T via PE."""
        pt = mm.tile([128, 512], BF, tag="mm", name="tp")
        nc.tensor.transpose(pt[:, 0:128], src_ap, ident)
        nc.vector.tensor_copy(out=dst_ap, in_=pt[:, 0:128])

    def rope(dst_bf, src_ap, st):
        """dst[128, DIM] bf16 = rope(src[128, DIM] bf16) for s-tile slot st."""
        rot = tmp.tile([128, DIM], BF, tag="scr1", bufs=1, name="rot")
        sp = src_ap.rearrange("p (n two) -> p n two", two=2)
        rp = rot.rearrange("p (n two) -> p n two", two=2)
        nc.vector.tensor_copy(out=rp[:, :, 0], in_=sp[:, :, 1])
        nc.vector.tensor_copy(out=rp[:, :, 1], in_=sp[:, :, 0])
        csb = cos_s[:, st, :].unsqueeze(1).broadcast_to([128, NH, HD])
        snb = sin_s[:, st, :].unsqueeze(1).broadcast_to([128, NH, HD])
        sv = src_ap.rearrange("p (nh e) -> p nh e", e=HD)
        nc.vector.tensor_mul(out=dst_bf.rearrange("p (nh e) -> p nh e", e=HD),
                             in0=sv, in1=csb)
        nc.vector.tensor_mul(out=rot.rearrange("p (nh e) -> p nh e", e=HD),
                             in0=rot.rearrange("p (nh e) -> p nh e", e=HD),
                             in1=snb)
        nc.vector.tensor_add(out=dst_bf, in0=dst_bf, in1=rot)

    wqkv_sb = {}       # layer -> [128, DC, 3*DIM] sbuf tile
    wo_sb = {}         # layer -> [128, DC, DIM]
    w2_sb = {}         # layer -> [128, HC, DIM]
    kv_out_t = {}      # (layer, st) -> shared AG output

    def load_wqkv(l):
        wt = wq_p.tile([128, DC, 3 * DIM], BF, tag="wqkv", bufs=1, name=f"wqkv{l}")
        nc.sync.dma_start(
            out=wt, in_=io["wqkv"][l].rearrange("(dc p) c -> p dc c", p=128))
        wqkv_sb[l] = wt

    def load_wo_w2(l):
        wo = wq_p.tile([128, DC, DIM], BF, tag="wo", bufs=1, name=f"wo{l}")
        nc.sync.dma_start(
            out=wo, in_=io["wot"][l].rearrange("(dc p) c -> p dc c", p=128))
        wo_sb[l] = wo
        w2 = wq_p.tile([128, HC, DIM], BF, tag="w2", bufs=1, name=f"w2{l}")
        nc.sync.dma_start(
            out=w2, in_=io["w2t"][l].rearrange("(hc p) c -> p hc c", p=128))
        w2_sb[l] = w2

    def qkv_gen(l, st):
        """qkv for (layer l, slot st) + combined K+V AllGather launch."""
        sl = slice(128 * st, 128 * (st + 1))
        xhT = tmpb.tile([128, DC, 128], BF, tag="xhT", bufs=1, name=f"xhT{l}_{st}")
        xhb = tmpb.tile([128, DIM], BF, tag="qkn", bufs=1, name=f"xhb{l}_{st}")
        rmsnorm_cast(xhb, h[:, st, :])
        yield
        for dc in range(DC):
            transpose128(xhT[:, dc, :], xhb[:, 128 * dc:128 * (dc + 1)])
        yield
        qkn = tmpb.tile([128, DIM], BF, tag="qkn", bufs=1, name=f"qkn{l}_{st}")
        vps = vpad[:, st, :].rearrange("p (nh e) -> p nh e", e=65)[:, :, 0:64]

        def mm_chunk(c0, cw):
            pt = mm.tile([128, 512], F32, tag="mm", name=f"qkvp{l}_{st}_{c0}")
            for dc in range(DC):
                nc.tensor.matmul(pt[:, :cw], xhT[:, dc, :],
                                 wqkv_sb[l][:, dc, c0:c0 + cw],
                                 start=(dc == 0), stop=(dc == DC - 1))
            if c0 >= 1536:  # V chunk: write strided into vpad (ones survive)
                h0 = (c0 - 1536) // 64
                nc.vector.tensor_copy(
                    out=vps[:, h0:h0 + cw // 64, :],
                    in_=pt[:, :cw].rearrange("p (nh e) -> p nh e", e=64))
            else:
                nc.vector.tensor_copy(out=qkn[:, c0 % DIM:c0 % DIM + cw],
                                      in_=pt[:, :cw])

        for (c0, cw) in QKV_CHUNKS[0:2]:   # K
            mm_chunk(c0, cw)
        yield
        kr = tmpb.tile([128, DIM], BF, tag="qkr", bufs=1, name=f"kr{l}_{st}")
        rope(kr, qkn[:, 0:DIM], st)
        for dc in range(DC):
            transpose128(koT[:, dc, sl], kr[:, 128 * dc:128 * (dc + 1)])
        yield
        for (c0, cw) in QKV_CHUNKS[2:4]:   # V
            mm_chunk(c0, cw)
        yield

        # combined K^T + V bounce + AllGather
        kv_in = dram.tile([KVW], BF, tag="kv_in", name=f"kvi{l}_{st}")
        nc.sync.dma_start(
            out=kv_in[0:DIM * 128].rearrange("(dc p s) -> p dc s", p=128, s=128),
            in_=koT[:, :, sl])
        nc.sync.dma_start(
            out=kv_in[DIM * 128:].rearrange("(p w) -> p w", p=128),
            in_=vpad[:, st, :])
        kv_out = dram.tile([NCORES, KVW], BF, tag="kv_out",
                           addr_space="Shared", name=f"kvo{l}_{st}")
        nc.gpsimd.collective_compute("AllGather", mybir.AluOpType.bypass,
                                     replica_groups=RG,
                                     ins=[kv_in.opt()], outs=[kv_out.opt()])
        kv_out_t[(l, st)] = kv_out

        # Q chunks + rope + transpose (off the AG critical path)
        for (c0, cw) in QKV_CHUNKS[4:6]:   # Q
            mm_chunk(c0, cw)
        yield
        qr = tmpb.tile([128, DIM], BF, tag="qkr", bufs=1, name=f"qr{l}_{st}")
        rope(qr, qkn[:, 0:DIM], st)
        for dc in range(DC):
            transpose128(qT[:, dc, sl], qr[:, 128 * dc:128 * (dc + 1)])

    def unbounce(l, st):
        """Scatter AG(l, st) output into kTf + v_aug (rank-major v index)."""
        kv_out = kv_out_t.pop((l, st))
        # kTf cols for rank r live at 256*r + 128*st (4D DMA unsupported)
        for r in range(NCORES):
            nc.sync.dma_start(
                out=kTf[:, :, 256 * r + 128 * st:256 * r + 128 * (st + 1)],
                in_=kv_out[r, 0:DIM * 128].rearrange("(dc p s) -> p dc s",
                                                     p=128, s=128))
        # v_aug block index: rank-major [st*8 + r]
        nc.sync.dma_start(
            out=v_aug[:, 8 * st:8 * (st + 1), :],
            in_=kv_out[:, DIM * 128:].rearrange("r (p w) -> p r w", p=128))

    def epilogue(ov_ap, l, st, hh):
        dch, offh = divmod(hh, 2)
        off = 64 * offh
        sl = slice(128 * st, 128 * (st + 1))
        rbc = tmp.tile([64, 128], F32, tag="rbc", bufs=2, name=f"rbc{l}_{st}_{hh}")
        nc.vector.reciprocal(out=rbc[0:1, :], in_=ov_ap[64:65, 0:128])
        nc.gpsimd.partition_broadcast(out_ap=rbc, in_ap=rbc[0:1, :])
        nc.vector.tensor_mul(out=oT[off:off + 64, dch, sl],
                             in0=ov_ap[0:64, 0:128], in1=rbc)

    def attn_gen(l, st):
        """Attention + wo + residual for (layer l, slot st). Yields per head
        so the emitter can interleave independent PE work into its bubbles."""
        nkt = 8 if st == 0 else 16
        sl = slice(128 * st, 128 * (st + 1))
        m0e = m0.rearrange("p (kt two s) -> p kt two s", two=2, s=128)
        for hh in range(NH):
            dch, offh = divmod(hh, 2)
            off = 64 * offh
            ov = ovp.tile([128, 512], F32, tag="ov", name=f"ov{l}_{st}_{hh}")
            for u in range(nkt // 8):
                sp = sc.tile([128, 1024], F32, tag="sc",
                             name=f"sc{l}_{st}_{hh}_{u}")
                for ktl in range(8):
                    kt = 8 * u + ktl
                    col = 256 * kt if kt <= 7 else 256 * (15 - kt) + 128
                    nc.tensor.matmul(
                        sp[:, 128 * ktl:128 * (ktl + 1)],
                        kTf[off:off + 64, dch, col:col + 128],
                        qT[off:off + 64, dch, sl],
                        start=True, stop=True)
                et = exp_p.tile([128, 1024], BF, tag="et", bufs=3,
                                name=f"et{l}_{st}_{hh}_{u}")
                nc.scalar.activation(out=et, in_=sp,
                                     func=mybir.ActivationFunctionType.Exp,
                                     scale=SCALE)
                if st == 0:
                    etv = et.rearrange("p (kt s) -> p kt s", s=128)
                    nc.vector.tensor_mul(out=etv, in0=etv, in1=m0e[:, :, 0, :])
                elif u == 1:
                    nc.vector.tensor_mul(out=et, in0=et, in1=m1[:, 0:1024])
                for ktl in range(8):
                    kt = 8 * u + ktl
                    vj = kt if kt <= 7 else 8 + (15 - kt)
                    nc.tensor.matmul(ov[0:65, 0:128],
                                     v_aug[:, vj, 65 * hh:65 * (hh + 1)],
                                     et[:, 128 * ktl:128 * (ktl + 1)],
                                     start=(kt == 0), stop=(kt == nkt - 1))
            epilogue(ov[:, 0:128], l, st, hh)
            yield

        # wo projection + residual for this slot's columns
        pts = {}
        for ci, (c0, cw) in enumerate(TWO_CHUNKS):
            pts[ci] = mm.tile([128, 512], F32, tag="mm", name=f"wop{l}_{st}_{ci}")
        for dc in range(DC):
            for ci, (c0, cw) in enumerate(TWO_CHUNKS):
                nc.tensor.matmul(pts[ci][:, :cw],
                                 oT[:, dc, sl],
                                 wo_sb[l][:, dc, c0:c0 + cw],
                                 start=(dc == 0), stop=(dc == DC - 1))
        for ci, (c0, cw) in enumerate(TWO_CHUNKS):
            nc.vector.tensor_add(out=h[:, st, c0:c0 + cw],
                                 in0=h[:, st, c0:c0 + cw],
                                 in1=pts[ci][:, :cw])

    w13_sb = {}

    def load_w13(l):
        wt = wb_p.tile([128, DC, 2 * HIDP], BF, tag="wbig", bufs=1, name=f"w13{l}")
        nc.sync.dma_start(
            out=wt, in_=io["w13"][l].rearrange("(dc p) c -> p dc c", p=128))
        w13_sb[l] = wt

    def ffn_gen(l, st):
        sl = slice(128 * st, 128 * (st + 1))
        yhT = tmpb.tile([128, DC, 128], BF, tag="xhT", bufs=1, name=f"yhT{l}_{st}")
        yhb = tmpb.tile([128, DIM], BF, tag="qkn", bufs=1, name=f"yhb{l}_{st}")
        rmsnorm_cast(yhb, h[:, st, :])
        yield
        for dc in range(DC):
            transpose128(yhT[:, dc, :], yhb[:, 128 * dc:128 * (dc + 1)])
        yield
        zbT = tmpb.tile([128, HC, 128], BF, tag="zbT", bufs=1, name=f"zbT{l}_{st}")
        for ck in range(4):
            p1 = mm.tile([128, 512], F32, tag="mm", name=f"z1p{l}_{st}_{ck}")
            p3 = mm.tile([128, 512], F32, tag="mm", name=f"z3p{l}_{st}_{ck}")
            for dc in range(DC):
                nc.tensor.matmul(p1, yhT[:, dc, :],
                                 w13_sb[l][:, dc, 512 * ck:512 * (ck + 1)],
                                 start=(dc == 0), stop=(dc == DC - 1))
                nc.tensor.matmul(p3, yhT[:, dc, :],
                                 w13_sb[l][:, dc, HIDP + 512 * ck:HIDP + 512 * (ck + 1)],
                                 start=(dc == 0), stop=(dc == DC - 1))
            sil = tmp.tile([128, 512], BF, tag="scr2", bufs=1,
                           name=f"sil{l}_{st}_{ck}")
            nc.scalar.activation(out=sil, in_=p1,
                                 func=mybir.ActivationFunctionType.Silu)
            zc = tmpb.tile([128, 512], BF, tag="zbc", name=f"zc{l}_{st}_{ck}")
            nc.vector.tensor_mul(out=zc, in0=sil, in1=p3)
            for j in range(4):
                transpose128(zbT[:, 4 * ck + j, :], zc[:, 128 * j:128 * (j + 1)])
            yield
        pts = {}
        for ci, (c0, cw) in enumerate(TWO_CHUNKS):
            pts[ci] = mm.tile([128, 512], F32, tag="mm", name=f"w2p{l}_{st}_{ci}")
        for hc in range(HC):
            for ci, (c0, cw) in enumerate(TWO_CHUNKS):
                nc.tensor.matmul(pts[ci][:, :cw],
                                 zbT[:, hc, :],
                                 w2_sb[l][:, hc, c0:c0 + cw],
                                 start=(hc == 0), stop=(hc == HC - 1))
            if hc % 4 == 3:
                yield
        for ci, (c0, cw) in enumerate(TWO_CHUNKS):
            nc.vector.tensor_add(out=h[:, st, c0:c0 + cw],
                                 in0=h[:, st, c0:c0 + cw],
                                 in1=pts[ci][:, :cw])

    # ---------- final norm / AG / output head helpers ----------
    hnT = sing.tile([128, DC, SLOC], BF, tag="koT", name="hnT")    # alias koT
    hnTf = sing.tile([128, DC, SEQ], BF, tag="kTf", name="hnTf")   # alias kTf
    hf_out_t = {}

    def final_gen(st):
        """Final rmsnorm + per-slot AllGather of h^T."""
        sl = slice(128 * st, 128 * (st + 1))
        fnb = tmpb.tile([128, DIM], BF, tag="qkn", bufs=1, name=f"fnb{st}")
        rmsnorm_cast(fnb, h[:, st, :])
        yield
        for dc in range(DC):
            transpose128(hnT[:, dc, sl], fnb[:, 128 * dc:128 * (dc + 1)])
        yield
        hf_in = dram.tile([DIM * 128], BF, tag="kv_in", name=f"hfi{st}")
        nc.sync.dma_start(
            out=hf_in.rearrange("(dc p s) -> p dc s", p=128, s=128),
            in_=hnT[:, :, sl])
        hf_out = dram.tile([NCORES, DIM * 128], BF, tag="kv_out",
                           addr_space="Shared", name=f"hfo{st}")
        nc.gpsimd.collective_compute("AllGather", mybir.AluOpType.bypass,
                                     replica_groups=RG,
                                     ins=[hf_in.opt()], outs=[hf_out.opt()])
        hf_out_t[st] = hf_out

    def head_block(src_ap, row0, tag):
        """logits[row0:row0+128, :] from src_ap = h^T cols [128, DC, 128]."""
        for half in range(8):
            ldr = tmp.tile([128, 500], BF, tag="ldr", bufs=2,
                           name=f"ldr{tag}_{half}")
            for vq in range(1):
                vc = half
                pt = mm.tile([128, 512], F32, tag="mm", name=f"hd{tag}_{vc}")
                for dc in range(DC):
                    nc.tensor.matmul(pt[:, :500],
                                     src_ap[:, dc, :],
                                     owt_sb[:, dc, 500 * vc:500 * (vc + 1)],
                                     start=(dc == 0), stop=(dc == DC - 1))
                if vc % 2 == 0:
                    nc.vector.tensor_copy(out=ldr[:, 500 * vq:500 * (vq + 1)],
                                          in_=pt[:, :500])
                else:
                    nc.scalar.copy(out=ldr[:, 500 * vq:500 * (vq + 1)],
                                   in_=pt[:, :500])
            nc.sync.dma_start(out=io["logits"][row0:row0 + 128,
                                               500 * half:500 * (half + 1)],
                              in_=ldr)

    # ================= the pipelined program =================
    def drive(*gens):
        live = list(gens)
        while live:
            nxt = []
            for g in live:
                try:
                    next(g)
                    nxt.append(g)
                except StopIteration:
                    pass
            live = nxt

    load_wqkv(0)
    load_wo_w2(0)
    drive(qkv_gen(0, 0))
    drive(qkv_gen(0, 1))
    load_w13(0)

    def e_chain(l):
        yield from ffn_gen(l, 0)
        if l + 1 < NL_BUILD:
            yield from qkv_gen(l + 1, 0)
        else:
            yield from final_gen(0)

    for l in range(NL_BUILD):
        unbounce(l, 0)
        drive(attn_gen(l, 0))
        unbounce(l, 1)
        if l + 1 < NL_BUILD:
            load_wqkv(l + 1)
        # slot-L attention fills the PE bubbles of slot-E's ffn+qkv chain
        drive(e_chain(l), attn_gen(l, 1))
        drive(ffn_gen(l, 1))
        if l + 1 < NL_BUILD:
            load_w13(l + 1)
            load_wo_w2(l + 1)
            drive(qkv_gen(l + 1, 1))
        else:
            drive(final_gen(1))

    # owt load (after last w13 use frees the pool slot)
    owt_sb = wb_p.tile([128, DC, VSH], BF, tag="wbig", bufs=1, name="owt")
    nc.sync.dma_start(
        out=owt_sb, in_=io["owt"].rearrange("(dc p) c -> p dc c", p=128))

    # un-bounce the final AGs and run the head over all 16 s-tiles.
    for st in (0, 1):
        hf_out = hf_out_t.pop(st)
        for r in range(NCORES):
            nc.sync.dma_start(
                out=hnTf[:, :, 256 * r + 128 * st:256 * r + 128 * (st + 1)],
                in_=hf_out[r].rearrange("(dc p s) -> p dc s", p=128, s=128))
        for r in range(NCORES):
            st_out = r if st == 0 else 15 - r
            head_block(hnTf[:, :, 256 * r + 128 * st:256 * r + 128 * (st + 1)],
                       128 * st_out, f"{r}_{st}")
    ctx.close()


_NC_CACHE = None


def _get_program():
    global _NC_CACHE
    if _NC_CACHE is None:
        _NC_CACHE = _build_program()
    return _NC_CACHE


def _prep_inputs(tokens, emb, wq, wk, wv, wo, w1, w2, w3,
                 attn_norm_w, ffn_norm_w, norm_w, out_w):
    f32 = np.float32
    tokens = np.asarray(tokens)
    emb = np.asarray(emb, f32)
    wq, wk, wv, wo = (np.asarray(a, f32) for a in (wq, wk, wv, wo))
    w1, w2, w3 = (np.asarray(a, f32) for a in (w1, w2, w3))
    attn_norm_w = np.asarray(attn_norm_w, f32)
    ffn_norm_w = np.asarray(ffn_norm_w, f32)
    norm_w = np.asarray(norm_w, f32)
    out_w = np.asarray(out_w, f32)

    # shared (rank-independent) weight transforms
    wqkv_l, w13_l, w2t_l, wot_l = [], [], [], []
    for l in range(NL):
        qkvT = np.concatenate([wq[l].T, wk[l].T, wv[l].T], axis=1)
        wqkv_l.append(qkvT * attn_norm_w[l][:, None])
        w1p = np.zeros((DIM, HIDP), f32)
        w3p = np.zeros((DIM, HIDP), f32)
        w1p[:, :HIDDEN] = w1[l].T
        w3p[:, :HIDDEN] = w3[l].T
        w13_l.append(np.concatenate([w1p, w3p], axis=1) * ffn_norm_w[l][:, None])
        w2p = np.zeros((HIDP, DIM), f32)
        w2p[:HIDDEN, :] = w2[l].T
        w2t_l.append(w2p)
        wot_l.append(wo[l].T)
    wqkv_np = np.stack(wqkv_l).astype(BF16NP)
    w13_np = np.stack(w13_l).astype(BF16NP)
    w2t_np = np.stack(w2t_l).astype(BF16NP)
    wot_np = np.stack(wot_l).astype(BF16NP)
    out_w_n = out_w * norm_w[None, :]

    freqs = THETA ** (-np.arange(0, HD, 2, dtype=f32) / HD)  # [32]
    kl = np.arange(128)[:, None]
    ql = np.arange(128)[None, :]

    in_maps = []
    for c in range(NCORES):
        tiles = (c, 15 - c)
        cols = np.concatenate([np.arange(128 * t, 128 * (t + 1)) for t in tiles])
        tok_own = np.asarray(tokens).reshape(-1)[cols].astype(np.int64)
        x0 = emb[tok_own].astype(f32)

        pos = cols.astype(f32)
        ang = pos[:, None] * freqs[None, :]          # [256, 32]
        cosp = np.cos(ang).astype(f32)
        sinp = np.sin(ang).astype(f32)
        cosn = np.repeat(cosp, 2, axis=1).astype(BF16NP)
        sinn = np.stack([-sinp, sinp], axis=2).reshape(SLOC, HD).astype(BF16NP)

        def make_mask(qtile, kt0, nkt):
            blocks = [(128 * kt + kl <= 128 * qtile + ql)
                      for kt in range(kt0, kt0 + nkt)]
            return np.concatenate(blocks, axis=1).astype(BF16NP)

        def make_mask0c(qtile):
            ones = np.ones((128, 128), bool)
            blocks = []
            for kt in range(8):
                blocks.append(128 * kt + kl <= 128 * qtile + ql)
                blocks.append(ones)
            return np.concatenate(blocks, axis=1).astype(BF16NP)

        owt_np = np.ascontiguousarray(
            out_w_n[VSH * c:VSH * (c + 1), :].T).astype(BF16NP)

        in_maps.append({
            "x0": x0,
            "wqkv": wqkv_np, "wot": wot_np, "w13": w13_np, "w2t": w2t_np,
            "owt": owt_np,
            "cosn": cosn, "sinn": sinn,
            "mask0": make_mask0c(c), "mask1": make_mask(15 - c, 8, 8),
        })
    return in_maps


def kernel(**inputs):
    nc = _get_program()
    in_maps = _prep_inputs(**inputs)
    res = run_bass_kernel_spmd(nc, in_maps, core_ids=list(range(NCORES)))
    shards = [np.asarray(res.results[c]["logits"], dtype=np.float32)
              for c in range(NCORES)]
    full = np.concatenate(shards, axis=1)  # [2048, 32000]
    return full[None]
